# revision 16
# baseline (speedup 1.0000x reference)
"""Trainium2 Bass kernel for BotanHadamardTransform: y = x @ H, with
x [4, 4096, 4096] f32 and H = H_4096/64 the normalized Sylvester
Hadamard matrix.

Factorization: H_4096 = H_32 (x) H_128.  For row-blocks v[32, 128]:
y = (FWHT_32 over the block axis) then (per-block @ H_128/64).  The
five radix-2 FWHT stages commute; we run distances 8, 4, 2 on DVE
(bf16, 2x packed mode) and FOLD distances 16 and 1 into the PE as a
4-term accumulation with +-H_128 stationaries:

  y[c]  (c = base + o0 + 16*o4, base in {0,2,..,14})
      = H^T sum_{s0,s4} (-1)^(o0 s0 + o4 s4) u3[base + s0 + 16 s4]

Why this shape (all from measurement on this part):
  - GpSimd tensor ops run ~1.92 ns/elem and CONTEND with DVE's 2x mode
    for the shared SBUF port pair: running both is slower than DVE
    alone (0.55 ns/elem).  So the butterfly is DVE-only, 3 stages.
  - LDWEIGHTS costs ~97 ns and is not hidden; with only two stationary
    matrices (+H_128, -H_128) the PE does 16-matmul bursts per LDW
    pair.  Per 2-group burst: 32 mm + 2 LDW.
  - Everything bf16 on the wire (host casts; tol 2e-2 vs ~5e-3 here):
    halves DMA (the ~106us floor) and doubles DVE rate.

Layout per core (1/8 of rows): xT [4096, 2048] bf16 in, yT out.
r-tiles of R=256 columns; DMA-in per 16-chunk half so the butterfly
starts after 1 MB.  PSUM groups = m-pair -> 8 output chunks as two
4-chunk runs; ScalarE evicts to bf16, out-DMAs alternate DMA rings.
"""
import os
import sys

sys.path.insert(0, "/opt/trn_rl_repo")

import numpy as np
import ml_dtypes

import concourse.bass as bass  # noqa: F401
import concourse.tile as tile
from concourse import bacc, mybir
from concourse.bass_utils import run_bass_kernel_spmd

NP_BF16 = np.dtype(ml_dtypes.bfloat16)

N_CORES = 8
N = 4096            # hidden dim
ROWS = 4 * 4096     # total rows
RC = ROWS // N_CORES  # rows (columns of xT) per core = 2048

B = 128             # PE-contracted factor (Hb = H_128 / 64)
NCH = N // 128      # 32 chunks of 128 partitions (chunk == A-block)
R = 512             # moving columns per r-tile (matmul n=512)
HCH = 16            # chunks per DMA/butterfly half


def _build():
    nc = bacc.Bacc("TRN2", target_bir_lowering=False, debug=False,
                   num_devices=N_CORES)
    bf16 = mybir.dt.bfloat16
    f32 = mybir.dt.float32

    xT_ap = nc.dram_tensor("xT", [N, RC], bf16, kind="ExternalInput").ap()
    # [H_128/64 | -H_128/64], horizontally packed
    hb_ap = nc.dram_tensor("Hb2", [B, 2 * B], bf16,
                           kind="ExternalInput").ap()
    yT_ap = nc.dram_tensor("yT", [N, RC], bf16, kind="ExternalOutput").ap()

    xT_v = xT_ap.rearrange("(c p) r -> p c r", p=128)   # [128, NCH, RC]
    yT_v = yT_ap.rearrange("(c p) r -> p c r", p=128)

    n_rt = RC // R

    # fold sign structure: output o=(o4,o0), term t=(s4,s0);
    # sign = (-1)^(o0*s0 + o4*s4)
    OUTS = [(0, 0), (0, 1), (1, 0), (1, 1)]
    TERMS = [(0, 0), (0, 1), (1, 0), (1, 1)]
    PLUS = [(o, t) for o in OUTS for t in TERMS
            if (o[1] * t[1] + o[0] * t[0]) % 2 == 0]
    MINUS = [(o, t) for o in OUTS for t in TERMS
             if (o[1] * t[1] + o[0] * t[0]) % 2 == 1]

    with tile.TileContext(nc) as tc:
        with (
            tc.tile_pool(name="hbp", bufs=1) as hbp,
            tc.tile_pool(name="pin", bufs=3) as pinp,
            tc.tile_pool(name="pa", bufs=2) as pap,
            tc.tile_pool(name="pb", bufs=2) as pbp,
            tc.tile_pool(name="pmv", bufs=2) as pmvp,
            tc.tile_pool(name="pev", bufs=4) as pevp,
            tc.tile_pool(name="ps", bufs=2, space="PSUM") as psp,
        ):
            hb = hbp.tile([128, 2 * B], bf16, tag="hb")
            nc.sync.dma_start(hb[:], hb_ap[:, :])
            Hp = hb[:, 0:B]        # +H_128/64
            Hm = hb[:, B:2 * B]    # -H_128/64

            def stage4d(dst, src, w, nch):
                """Radix-2 stage, slab width w chunks over nch chunks:
                one DVE 4D op per add/sub (no GpSimd: shared-port)."""
                dv = dst.rearrange("p (g w) r -> p g w r", w=w)
                sv = src.rearrange("p (g w) r -> p g w r", w=w)
                lo = sv[:, 0::2, :, :]
                hi = sv[:, 1::2, :, :]
                nc.vector.tensor_add(dv[:, 0::2, :, :], lo, hi)
                nc.vector.tensor_sub(dv[:, 1::2, :, :], lo, hi)

            for it in range(n_rt):
                r0 = it * R
                mv = pmvp.tile([128, NCH, R], bf16, tag="mv",
                               name=f"mv_{it}")
                pbs = []
                for ih in range(2):      # 16-chunk halves
                    c0 = ih * HCH
                    xin = pinp.tile([128, HCH, R], bf16, tag="pin",
                                    name=f"xin_{it}_{ih}")
                    nc.sync.dma_start(
                        xin[:], xT_v[:, c0:c0 + HCH, r0:r0 + R])
                    # d8 within the half (pairs (c, c+8))
                    t1 = pap.tile([128, HCH, R], bf16, tag="pa",
                                  name=f"t1_{it}_{ih}")
                    stage4d(t1, xin, 8, HCH)
                    # d4 within the half
                    t2 = pbp.tile([128, HCH, R], bf16, tag="pb",
                                  name=f"t2_{it}_{ih}")
                    stage4d(t2, t1, 4, HCH)
                    # d2 within the half -> mv
                    stage4d(mv[:, c0:c0 + HCH, :], t2, 2, HCH)

                # PE: fold d16 and d1; per-m groups (outputs 2m, 2m+1,
                # 2m+16, 2m+17), +H burst (10 mm) then -H (6 mm)
                order = [("p", o, t) for (o, t) in PLUS] + \
                        [("m", o, t) for (o, t) in MINUS]
                first = {}
                last = {}
                for i, (ph, o, t) in enumerate(order):
                    if o not in first:
                        first[o] = i
                    last[o] = i
                for m in range(8):
                    pp = psp.tile([128, 4, R], f32, tag="ps",
                                  name=f"pp_{it}_{m}")
                    for i, (ph, o, t) in enumerate(order):
                        o4, o0 = o
                        s4, s0 = t
                        st = Hp if ph == "p" else Hm
                        nc.tensor.matmul(
                            pp[:, o4 * 2 + o0, :],
                            st,
                            mv[:, 2 * m + s0 + 16 * s4, :],
                            start=(first[o] == i),
                            stop=(last[o] == i),
                        )
                    ev = pevp.tile([128, 4, R], bf16, tag="pev",
                                   name=f"ev_{it}_{m}")
                    nc.scalar.copy(ev[:].rearrange("p c r -> p (c r)"),
                                   pp[:].rearrange("p c r -> p (c r)"))
                    eng = nc.sync if m % 2 == 0 else nc.scalar
                    eng.dma_start(
                        yT_v[:, 2 * m:2 * m + 2, r0:r0 + R],
                        ev[:, 0:2, :])
                    eng.dma_start(
                        yT_v[:, 2 * m + 16:2 * m + 18, r0:r0 + R],
                        ev[:, 2:4, :])

    nc.compile()
    return nc


_prog = None


def _get_prog():
    global _prog
    if _prog is None:
        _prog = _build()
    return _prog


def _run(xT, Hb2, trace=False):
    nc = _get_prog()
    in_maps = [
        {"xT": np.ascontiguousarray(xT[:, c * RC:(c + 1) * RC]),
         "Hb2": Hb2}
        for c in range(N_CORES)
    ]
    res = run_bass_kernel_spmd(nc, in_maps, core_ids=list(range(N_CORES)),
                               trace=trace)
    return res


def _make_hb2(H):
    Hb = np.asarray(H)[:B, :B]
    return np.ascontiguousarray(
        np.concatenate([Hb, -Hb], axis=1).astype(NP_BF16))


def kernel(x, H):
    x = np.asarray(x)
    H = np.asarray(H)
    xT = np.ascontiguousarray(
        x.reshape(ROWS, N).T.astype(NP_BF16))             # [N, ROWS] bf16
    Hb2 = _make_hb2(H)
    res = _run(xT, Hb2)
    y = np.empty((ROWS, N), dtype=np.float32)
    for c in range(N_CORES):
        y[c * RC:(c + 1) * RC, :] = res.results[c]["yT"].T.astype(np.float32)
    return y.reshape(4, 4096, N)
